# revision 51
# baseline (speedup 1.0000x reference)
"""Bass/Trainium2 kernel for nn_Bert_coss (8-core data-parallel over batch).

Computation (per example):
  o1 = relu(X1 @ W.T + b)            [S, H]
  o2 = relu(X2 @ W.T + b)            [S, H]
  o1_doc, o2_doc = mean over S       [H]
  out = sigmoid(relu(concat(o1_doc, o2_doc) @ fd_w.T + fd_b) @ ff_w.T + ff_b)
  scores[s] = o1e[s] . o2_doc   (o1e = o1 ++ o1_doc row), s in 0..S
  att = softmax(scores); output rows 0..S-1 = att[0:S], row S = out.

The reference's full [S+1,S+1] co-attention einsum is only consumed through
its last column, so only S+1 dot products against o2_doc are needed.

Precision/throughput split (calibrated on-HW: fp16 matmul ~507ns per
K256x128x512 block, fp8e4 DoubleRow ~292ns):
 - o1 path feeds the softmax scores directly and needs full precision:
   fp16 matmuls (12 x 512-row instructions per example).
 - o2 is only consumed through its S-mean doc vector: per-entry fp8 noise
   averages out over 512 positions and its shared component cancels in the
   softmax, so one fp8(E4M3) DoubleRow product (6 instructions, W pre-scaled
   by 64 into e4m3's range) suffices. Validated end-to-end ~1.5e-3 rel err
   vs the 2e-2 gate.
DMA drops to 3 bytes per (X1,X2) element pair; PE drops 25% vs all-fp16.

Evictions are split across engines to keep each under the DMA-bound
critical path: o1 relu+bias on ACT (accum_out = doc sums), o2 relu on DVE
as max(psum + 64b, 0) followed by an add-reduce (the 64 folds into the
doc-scale constants). Score softmax normalization runs on GpSimd.
"""

import sys

for _p in ("/opt/trn_rl_repo",):
    if _p not in sys.path:
        sys.path.append(_p)

import numpy as np
import ml_dtypes
from contextlib import ExitStack

import concourse.bass as bass
import concourse.tile as tile
from concourse import bacc, mybir
from concourse import bass_utils

B, S, V, H = 64, 512, 768, 256
NCORES = 8
BL = B // NCORES        # examples per core
KV = V // 128           # contraction chunks for the mlp matmul
KP = KV // 2            # DoubleRow k-pairs
MH = H // 128           # output-partition chunks of H
WSCALE = 64.0           # W pre-scale so fp8 e4m3 covers its range

F32 = mybir.dt.float32
F16 = mybir.dt.float16
F8 = mybir.dt.float8e4
AF = mybir.ActivationFunctionType
DR = mybir.MatmulPerfMode.DoubleRow
E4NP = ml_dtypes.float8_e4m3


def _build_kernel(tc):
    nc = tc.nc
    x1d = nc.dram_tensor("x1", [BL, 128, KV * S], F16,
                         kind="ExternalInput").ap()
    x2d = nc.dram_tensor("x2", [BL, 128, KV * S], F8,
                         kind="ExternalInput").ap()
    w16d = nc.dram_tensor("w16", [128, KV * H], F16, kind="ExternalInput").ap()
    w8d = nc.dram_tensor("w8", [128, KV * H], F8, kind="ExternalInput").ap()
    mlpb_d = nc.dram_tensor("mlpb", [128, MH], F32, kind="ExternalInput").ap()
    mlpb64_d = nc.dram_tensor("mlpb64", [128, MH], F32,
                              kind="ExternalInput").ap()
    fdw_d = nc.dram_tensor("fdw", [128, 4 * H], F16, kind="ExternalInput").ap()
    fdb_d = nc.dram_tensor("fdb", [128, MH], F32, kind="ExternalInput").ap()
    ffw_d = nc.dram_tensor("ffw", [128, MH], F16, kind="ExternalInput").ap()
    nffb_d = nc.dram_tensor("nffb", [1, 1], F32, kind="ExternalInput").ap()
    out = nc.dram_tensor("out", [BL, S + 1], F32, kind="ExternalOutput").ap()

    with ExitStack() as ctx:
        const = ctx.enter_context(tc.tile_pool(name="const", bufs=1))

        # PE-critical params ride the fast sync queue, interleaved with the
        # first example's X chunks (issued below) so the PE starts ~9.8us
        w16 = const.tile([128, KV * H], F16)
        w16_v = w16[:].rearrange("p (k h) -> p k h", k=KV)
        w8 = const.tile([128, KV * H], F8)
        w8_v = w8[:].rearrange("p (k h) -> p k h", k=KV)
        mlpb_sb = const.tile([128, MH], F32)
        mlpb64_sb = const.tile([128, MH], F32)
        fdw_sb = const.tile([128, 4 * H], F16)
        fdb_sb = const.tile([128, MH], F32)
        ffw_sb = const.tile([128, MH], F16)
        nffb_sb = const.tile([1, 1], F32)
        # dummy Exp so the ACT table set loads during the DMA ramp instead of
        # on the end-of-kernel critical path (scale=0 -> input value unused)
        expwarm = const.tile([1, 1], F32)
        nc.scalar.activation(expwarm[:], expwarm[:], AF.Exp, scale=0.0)


        def _late_const_dmas():
            # parameters only needed by the end-of-kernel head
            nc.scalar.dma_start(fdw_sb[:], fdw_d)
            nc.scalar.dma_start(fdb_sb[:], fdb_d)
            nc.scalar.dma_start(ffw_sb[:], ffw_d)
            nc.scalar.dma_start(nffb_sb[:], nffb_d)

        # doc-vector raw sums; column b*4 + c, c in (o1m0, o1m1, o2m0, o2m1)
        # o1 columns hold 512*o1_doc; o2 columns hold 512*64*o2_doc
        docs_all = const.tile([128, 4 * BL], F32)
        # true-scale doc vectors in fp16 (score matvec lhsT + head rhs)
        dscs = const.tile([128, 4 * BL], F16)

        with ExitStack() as mctx:
            x1pool = mctx.enter_context(tc.tile_pool(name="x1", bufs=4))
            x2pool = mctx.enter_context(tc.tile_pool(name="x2", bufs=4))
            o1pool = mctx.enter_context(tc.tile_pool(name="o1", bufs=3))
            o2pool = mctx.enter_context(tc.tile_pool(name="o2", bufs=2))
            apool = mctx.enter_context(tc.tile_pool(name="att", bufs=3))
            mm1_ps = mctx.enter_context(tc.tile_pool(name="mm1", bufs=2, space="PSUM"))
            mm2_ps = mctx.enter_context(tc.tile_pool(name="mm2", bufs=1, space="PSUM"))
            sc_ps = mctx.enter_context(tc.tile_pool(name="scps", bufs=1, space="PSUM"))
            dd_ps = mctx.enter_context(tc.tile_pool(name="ddps", bufs=1, space="PSUM"))

            def score_mms(b, o1T):
                ssc = sc_ps.tile([1, S], F32, tag="ssc", name="ssc")
                for hk in range(MH):
                    nc.tensor.matmul(
                        ssc[:],
                        dscs[:, b * 4 + 2 + hk : b * 4 + 3 + hk],
                        o1T[:, hk * S : (hk + 1) * S],
                        start=(hk == 0),
                        stop=(hk == MH - 1),
                    )
                sdd = dd_ps.tile([1, 1], F32, tag="sdd", name="sdd")
                for hk in range(MH):
                    nc.tensor.matmul(
                        sdd[:],
                        dscs[:, b * 4 + 2 + hk : b * 4 + 3 + hk],
                        dscs[:, b * 4 + hk : b * 4 + hk + 1],
                        start=(hk == 0),
                        stop=(hk == MH - 1),
                    )
                return ssc, sdd

            def score_post(b, ssc, sdd):
                # softmax on partition 0, straight from PSUM; no max-
                # subtraction (scores are O(25), far inside fp32 exp range)
                att = apool.tile([1, S], F32)
                s1 = apool.tile([1, 1], F32, name="s1")
                nc.scalar.activation(att[:], ssc[:], AF.Exp, accum_out=s1[:])
                edd = apool.tile([1, 1], F32, name="edd")
                nc.scalar.activation(edd[:], sdd[:], AF.Exp)
                stot = apool.tile([1, 1], F32, name="stot")
                nc.vector.tensor_add(stot[:], s1[:], edd[:])
                rs = apool.tile([1, 1], F32, name="rs")
                nc.vector.reciprocal(rs[:], stot[:])
                nc.vector.tensor_scalar_mul(att[:], att[:], rs[:])
                # SP HWDGE: att(b-2) is long ready when SP reaches this
                # trigger, so no sequencer stall (SWDGE's software ring is
                # slow and was adding ~3us of output tail)
                nc.sync.dma_start(out[b : b + 1, 0:S], att[:])

            def do_scores(b, o1T):
                ssc, sdd = score_mms(b, o1T)
                score_post(b, ssc, sdd)

            prevs = []
            for b in range(BL):
                x1t = x1pool.tile([128, KV * S], F16, tag="x1t", name="x1t")
                x1_v = x1t[:].rearrange("p (k s) -> p k s", k=KV)
                x2t = x2pool.tile([128, KV * S], F8, tag="x2t", name="x2t")
                x2_v = x2t[:].rearrange("p (k s) -> p k s", k=KV)
                if b == 0:
                    # startup schedule on one fast queue: weights and the
                    # first example's X interleaved so the PE starts on k0-1
                    # after ~2 small transfers and then chases the stream
                    x1src = x1d[b].rearrange("p (k s) -> p k s", k=KV)
                    nc.sync.dma_start(w16[:, 0 : 2 * H], w16d[:, 0 : 2 * H])
                    nc.sync.dma_start(x1_v[:, 0:2, :], x1src[:, 0:2, :])
                    nc.sync.dma_start(w16[:, 2 * H :], w16d[:, 2 * H :])
                    nc.sync.dma_start(x1_v[:, 2:4, :], x1src[:, 2:4, :])
                    nc.sync.dma_start(x1_v[:, 4:6, :], x1src[:, 4:6, :])
                    nc.sync.dma_start(w8[:], w8d)
                    # x2 rides the second (scalar) queue in parallel: the
                    # sync queue keeps a pure weights+x1 stream
                    nc.scalar.dma_start(mlpb_sb[:], mlpb_d)
                    nc.scalar.dma_start(mlpb64_sb[:], mlpb64_d)
                    nc.scalar.dma_start(x2t[:], x2d[b])
                else:
                    nc.sync.dma_start(x1t[:], x1d[b])
                    nc.scalar.dma_start(x2t[:], x2d[b])
                if b == 1:
                    _late_const_dmas()

                o1T = o1pool.tile([128, MH * S], F16)
                pss1 = [
                    mm1_ps.tile([128, S], F32, tag=f"p1{m}", name=f"p1{m}")
                    for m in range(MH)
                ]
                for k in range(KV):
                    for m in range(MH):
                        nc.tensor.matmul(
                            pss1[m][:],
                            w16_v[:, k, m * 128 : (m + 1) * 128],
                            x1_v[:, k, :],
                            start=(k == 0),
                            stop=(k == KV - 1),
                        )

                # the previous-previous example's score matvecs interleave
                # into this example's o2 block: no PE boundary stall, and
                # their inputs have been ready for a full example
                sc_prev = prevs.pop(0) if len(prevs) >= 2 else None
                if sc_prev is not None:
                    pb, po1T = sc_prev
                    ssc = sc_ps.tile([1, S], F32, tag="ssc", name="ssc")
                    sdd = dd_ps.tile([1, 1], F32, tag="sdd", name="sdd")
                pss2 = [
                    mm2_ps.tile([128, S], F32, tag=f"p2{m}", name=f"p2{m}")
                    for m in range(MH)
                ]
                for j in range(KP):
                    for m in range(MH):
                        nc.tensor.matmul(
                            pss2[m][:],
                            w8_v[:, 2 * j : 2 * j + 2, m * 128 : (m + 1) * 128],
                            x2_v[:, 2 * j : 2 * j + 2, :],
                            start=(j == 0),
                            stop=(j == KP - 1),
                            perf_mode=DR,
                        )
                    if sc_prev is not None and j < MH:
                        nc.tensor.matmul(
                            ssc[:],
                            dscs[:, pb * 4 + 2 + j : pb * 4 + 3 + j],
                            po1T[:, j * S : (j + 1) * S],
                            start=(j == 0),
                            stop=(j == MH - 1),
                        )
                if sc_prev is not None:
                    for hk in range(MH):
                        nc.tensor.matmul(
                            sdd[:],
                            dscs[:, pb * 4 + 2 + hk : pb * 4 + 3 + hk],
                            dscs[:, pb * 4 + hk : pb * 4 + hk + 1],
                            start=(hk == 0),
                            stop=(hk == MH - 1),
                        )
                for m in range(MH):
                    nc.scalar.activation(
                        o1T[:, m * S : (m + 1) * S],
                        pss1[m][:],
                        AF.Relu,
                        bias=mlpb_sb[:, m : m + 1],
                        accum_out=docs_all[:, b * 4 + m : b * 4 + m + 1],
                    )
                for m in range(MH):
                    # relu(p/64 + b) * 64 = max(p + 64b, 0); the 64 folds into
                    # the dscs scaling below. Only the doc sum is consumed.
                    # On the last example m0 goes to ACT so the two tail
                    # evictions run in parallel across engines.
                    if b == BL - 1 and m == 0:
                        o2scr = o2pool.tile([128, S], F16, tag="o2scr",
                                            name="o2scr")
                        nc.scalar.activation(
                            o2scr[:],
                            pss2[m][:],
                            AF.Relu,
                            bias=mlpb_sb[:, m : m + 1],
                            scale=1.0 / WSCALE,
                            accum_out=docs_all[:, b * 4 + 2 : b * 4 + 3],
                        )
                        # ACT path wrote the true-scale sum; scale it as o1
                        nc.vector.tensor_scalar_mul(
                            dscs[:, b * 4 + 2 : b * 4 + 3],
                            docs_all[:, b * 4 + 2 : b * 4 + 3], 1.0 / S)
                        continue
                    o2scr = o2pool.tile([128, S], F16, tag="o2scr", name="o2scr")
                    nc.vector.tensor_scalar(
                        o2scr[:],
                        pss2[m][:],
                        mlpb64_sb[:, m : m + 1],
                        0.0,
                        mybir.AluOpType.add,
                        mybir.AluOpType.max,
                    )
                    nc.vector.tensor_reduce(
                        docs_all[:, b * 4 + 2 + m : b * 4 + 3 + m],
                        o2scr[:],
                        mybir.AxisListType.X,
                        mybir.AluOpType.add,
                    )
                    nc.vector.tensor_scalar_mul(
                        dscs[:, b * 4 + 2 + m : b * 4 + 3 + m],
                        docs_all[:, b * 4 + 2 + m : b * 4 + 3 + m],
                        1.0 / (S * WSCALE))
                nc.vector.tensor_scalar_mul(
                    dscs[:, b * 4 : b * 4 + 2],
                    docs_all[:, b * 4 : b * 4 + 2], 1.0 / S)
                if sc_prev is not None:
                    score_post(pb, ssc, sdd)
                prevs.append((b, o1T))
            # ---- tail: final scores interleaved with the head ----
            # head columns 0..BL-2 only need the first BL-1 doc vectors, so
            # they run before ex BL-1's eviction chain completes; their ph
            # psums reuse the (long-evicted) o1 psum banks.
            hpool = mctx.enter_context(tc.tile_pool(name="head", bufs=1))
            docs_v = dscs[:].rearrange("p (b k) -> p k b", k=4)
            fdw_v = fdw_sb[:].rearrange("p (k h) -> p k h", k=4)
            h16 = hpool.tile([128, MH * BL], F16)

            def head_cols(lo, hi):
                for m in range(MH):
                    ph = mm1_ps.tile([128, hi - lo], F32, tag=f"p1{m}",
                                     name=f"ph{m}")
                    for kc in range(4):
                        nc.tensor.matmul(
                            ph[:],
                            fdw_v[:, kc, m * 128 : (m + 1) * 128],
                            docs_v[:, kc, lo:hi],
                            start=(kc == 0),
                            stop=(kc == 3),
                        )
                    nc.scalar.activation(
                        h16[:, m * BL + lo : m * BL + hi],
                        ph[:],
                        AF.Relu,
                        bias=fdb_sb[:, m : m + 1],
                    )

            do_scores(*prevs.pop(0))
            head_cols(0, BL - 1)
            do_scores(*prevs.pop(0))
            head_cols(BL - 1, BL)

            po = dd_ps.tile([1, BL], F32, tag="sdd", name="po")
            for m in range(MH):
                nc.tensor.matmul(
                    po[:],
                    ffw_sb[:, m : m + 1],
                    h16[:, m * BL : (m + 1) * BL],
                    start=(m == 0),
                    stop=(m == MH - 1),
                )
            # sigmoid(x) = 1/(1+exp(-x)) — stays in the Exp table set
            sig_row = hpool.tile([1, BL], F32)
            nc.scalar.activation(sig_row[:], po[:], AF.Exp,
                                 bias=nffb_sb[0:1, 0:1], scale=-1.0)
            nc.vector.tensor_scalar_add(sig_row[:], sig_row[:], 1.0)
            nc.vector.reciprocal(sig_row[:], sig_row[:])

            # final output column: out[:, S] = sigmoid head values
            nc.sync.dma_start(
                out[:, S : S + 1],
                sig_row[0:1, :].rearrange("o (b s) -> o b s", b=BL),
            )


_NC_CACHE = None


def _get_nc():
    global _NC_CACHE
    if _NC_CACHE is None:
        nc = bacc.Bacc("TRN2", target_bir_lowering=False, debug=False,
                       num_devices=NCORES)
        with tile.TileContext(nc) as tc:
            _build_kernel(tc)
        nc.compile()
        _NC_CACHE = nc
    return _NC_CACHE


def kernel(output_1, output_2, mlp_w, mlp_b, fd_w, fd_b, ff_w, ff_b):
    output_1 = np.asarray(output_1, dtype=np.float32)
    output_2 = np.asarray(output_2, dtype=np.float32)
    mlp_w = np.asarray(mlp_w, dtype=np.float32)
    mlp_b = np.asarray(mlp_b, dtype=np.float32)
    fd_w = np.asarray(fd_w, dtype=np.float32)
    fd_b = np.asarray(fd_b, dtype=np.float32)
    ff_w = np.asarray(ff_w, dtype=np.float32)
    ff_b = np.asarray(ff_b, dtype=np.float32)

    # shard over batch; [*, S, V] -> [*, p, k, S] with V = k*128 + p
    def to_pks(x):
        return np.ascontiguousarray(
            x.reshape(NCORES, BL, S, KV, 128).transpose(0, 1, 4, 3, 2))

    x1 = to_pks(output_1).astype(np.float16).reshape(NCORES, BL, 128, KV * S)
    x2 = to_pks(output_2).astype(E4NP).reshape(NCORES, BL, 128, KV * S)

    wt = np.ascontiguousarray(
        mlp_w.T.reshape(KV, 128, H).transpose(1, 0, 2))        # [p, k, H]
    w16 = wt.astype(np.float16).reshape(128, KV * H)
    w8 = (wt * WSCALE).astype(E4NP).reshape(128, KV * H)

    mlpb = np.ascontiguousarray(mlp_b.reshape(MH, 128).T)      # [128, MH]
    mlpb64 = np.ascontiguousarray(mlpb * WSCALE)
    fdw = np.ascontiguousarray(
        fd_w.T.reshape(4, 128, H).transpose(1, 0, 2)
    ).reshape(128, 4 * H).astype(np.float16)
    fdb = np.ascontiguousarray(fd_b.reshape(MH, 128).T)
    ffw = np.ascontiguousarray(
        ff_w.reshape(MH, 128).T).astype(np.float16)            # [128, MH]
    nffb = np.ascontiguousarray(-ff_b.reshape(1, 1))

    in_maps = [
        dict(x1=x1[c], x2=x2[c], w16=w16, w8=w8, mlpb=mlpb, mlpb64=mlpb64,
             fdw=fdw, fdb=fdb, ffw=ffw, nffb=nffb)
        for c in range(NCORES)
    ]
    global _LAST_IN_MAPS
    _LAST_IN_MAPS = in_maps
    nc = _get_nc()
    res = bass_utils.run_bass_kernel_spmd(nc, in_maps, core_ids=list(range(NCORES)))
    att = np.concatenate([res.results[c]["out"] for c in range(NCORES)], axis=0)
    return np.ascontiguousarray(att.T)  # [S+1, B]


# revision 53
# speedup vs baseline: 1.1982x; 1.1982x over previous
"""Bass/Trainium2 kernel for nn_Bert_coss (8-core data-parallel over batch).

Computation (per example):
  o1 = relu(X1 @ W.T + b)            [S, H]
  o2 = relu(X2 @ W.T + b)            [S, H]
  o1_doc, o2_doc = mean over S       [H]
  out = sigmoid(relu(concat(o1_doc, o2_doc) @ fd_w.T + fd_b) @ ff_w.T + ff_b)
  scores[s] = o1e[s] . o2_doc   (o1e = o1 ++ o1_doc row), s in 0..S
  att = softmax(scores); output rows 0..S-1 = att[0:S], row S = out.

The reference's full [S+1,S+1] co-attention einsum is only consumed through
its last column, so only S+1 dot products against o2_doc are needed.

Precision/throughput split (calibrated on-HW: fp16 matmul ~507ns per
K256x128x512 block, fp8e4 DoubleRow ~292ns):
 - o1 path feeds the softmax scores directly and needs full precision:
   fp16 matmuls (12 x 512-row instructions per example).
 - o2 is only consumed through its S-mean doc vector: per-entry fp8 noise
   averages out over 512 positions and its shared component cancels in the
   softmax, so one fp8(E4M3) DoubleRow product (6 instructions, W pre-scaled
   by 64 into e4m3's range) suffices. Validated end-to-end ~1.5e-3 rel err
   vs the 2e-2 gate.
DMA drops to 3 bytes per (X1,X2) element pair; PE drops 25% vs all-fp16.

Evictions are split across engines to keep each under the DMA-bound
critical path: o1 relu+bias on ACT (accum_out = doc sums), o2 relu on DVE
as max(psum + 64b, 0) followed by an add-reduce (the 64 folds into the
doc-scale constants). Score softmax normalization runs on GpSimd.
"""

import sys

for _p in ("/opt/trn_rl_repo",):
    if _p not in sys.path:
        sys.path.append(_p)

import numpy as np
import ml_dtypes
from contextlib import ExitStack

import concourse.bass as bass
import concourse.tile as tile
from concourse import bacc, mybir
from concourse import bass_utils

B, S, V, H = 64, 512, 768, 256
NCORES = 8
BL = B // NCORES        # examples per core
KV = V // 128           # contraction chunks for the mlp matmul
KP = KV // 2            # DoubleRow k-pairs
MH = H // 128           # output-partition chunks of H
WSCALE = 64.0           # W pre-scale so fp8 e4m3 covers its range

F32 = mybir.dt.float32
F16 = mybir.dt.float16
F8 = mybir.dt.float8e4
AF = mybir.ActivationFunctionType
DR = mybir.MatmulPerfMode.DoubleRow
E4NP = ml_dtypes.float8_e4m3


def _build_kernel(tc):
    nc = tc.nc
    x1d = nc.dram_tensor("x1", [BL, 128, KV * S], F16,
                         kind="ExternalInput").ap()
    x2d = nc.dram_tensor("x2", [BL, 128, KV * S], F8,
                         kind="ExternalInput").ap()
    w16d = nc.dram_tensor("w16", [128, KV * H], F16, kind="ExternalInput").ap()
    w8d = nc.dram_tensor("w8", [128, KV * H], F8, kind="ExternalInput").ap()
    mlpb_d = nc.dram_tensor("mlpb", [128, MH], F32, kind="ExternalInput").ap()
    mlpb64_d = nc.dram_tensor("mlpb64", [128, MH], F32,
                              kind="ExternalInput").ap()
    fdw_d = nc.dram_tensor("fdw", [128, 4 * H], F16, kind="ExternalInput").ap()
    fdb_d = nc.dram_tensor("fdb", [128, MH], F32, kind="ExternalInput").ap()
    ffw_d = nc.dram_tensor("ffw", [128, MH], F16, kind="ExternalInput").ap()
    nffb_d = nc.dram_tensor("nffb", [1, 1], F32, kind="ExternalInput").ap()
    out = nc.dram_tensor("out", [BL, S + 1], F32, kind="ExternalOutput").ap()

    with ExitStack() as ctx:
        const = ctx.enter_context(tc.tile_pool(name="const", bufs=1))

        # PE-critical params ride the fast sync queue, interleaved with the
        # first example's X chunks (issued below) so the PE starts ~9.8us
        w16 = const.tile([128, KV * H], F16)
        w16_v = w16[:].rearrange("p (k h) -> p k h", k=KV)
        w8 = const.tile([128, KV * H], F8)
        w8_v = w8[:].rearrange("p (k h) -> p k h", k=KV)
        mlpb_sb = const.tile([128, MH], F32)
        mlpb64_sb = const.tile([128, MH], F32)
        fdw_sb = const.tile([128, 4 * H], F16)
        fdb_sb = const.tile([128, MH], F32)
        ffw_sb = const.tile([128, MH], F16)
        nffb_sb = const.tile([1, 1], F32)
        # dummy Exp so the ACT table set loads during the DMA ramp instead of
        # on the end-of-kernel critical path (scale=0 -> input value unused)
        expwarm = const.tile([1, 1], F32)
        nc.scalar.activation(expwarm[:], expwarm[:], AF.Exp, scale=0.0)


        def _late_const_dmas():
            # parameters only needed by the end-of-kernel head
            nc.scalar.dma_start(fdw_sb[:], fdw_d)
            nc.scalar.dma_start(fdb_sb[:], fdb_d)
            nc.scalar.dma_start(ffw_sb[:], ffw_d)
            nc.scalar.dma_start(nffb_sb[:], nffb_d)

        # doc-vector raw sums; column b*4 + c, c in (o1m0, o1m1, o2m0, o2m1)
        # o1 columns hold 512*o1_doc; o2 columns hold 512*64*o2_doc
        docs_all = const.tile([128, 4 * BL], F32)
        # true-scale doc vectors in fp16 (score matvec lhsT + head rhs)
        dscs = const.tile([128, 4 * BL], F16)

        with ExitStack() as mctx:
            x1pool = mctx.enter_context(tc.tile_pool(name="x1", bufs=4))
            x2pool = mctx.enter_context(tc.tile_pool(name="x2", bufs=4))
            o1pool = mctx.enter_context(tc.tile_pool(name="o1", bufs=3))
            o2pool = mctx.enter_context(tc.tile_pool(name="o2", bufs=2))
            apool = mctx.enter_context(tc.tile_pool(name="att", bufs=3))
            mm1_ps = mctx.enter_context(tc.tile_pool(name="mm1", bufs=2, space="PSUM"))
            mm2_ps = mctx.enter_context(tc.tile_pool(name="mm2", bufs=1, space="PSUM"))
            sc_ps = mctx.enter_context(tc.tile_pool(name="scps", bufs=1, space="PSUM"))
            dd_ps = mctx.enter_context(tc.tile_pool(name="ddps", bufs=1, space="PSUM"))

            def score_mms(b, o1T):
                ssc = sc_ps.tile([1, S], F32, tag="ssc", name="ssc")
                for hk in range(MH):
                    nc.tensor.matmul(
                        ssc[:],
                        dscs[:, b * 4 + 2 + hk : b * 4 + 3 + hk],
                        o1T[:, hk * S : (hk + 1) * S],
                        start=(hk == 0),
                        stop=(hk == MH - 1),
                    )
                sdd = dd_ps.tile([1, 1], F32, tag="sdd", name="sdd")
                for hk in range(MH):
                    nc.tensor.matmul(
                        sdd[:],
                        dscs[:, b * 4 + 2 + hk : b * 4 + 3 + hk],
                        dscs[:, b * 4 + hk : b * 4 + hk + 1],
                        start=(hk == 0),
                        stop=(hk == MH - 1),
                    )
                return ssc, sdd

            def score_post(b, ssc, sdd):
                # softmax on partition 0, straight from PSUM; no max-
                # subtraction (scores are O(25), far inside fp32 exp range)
                att = apool.tile([1, S], F32)
                s1 = apool.tile([1, 1], F32, name="s1")
                nc.scalar.activation(att[:], ssc[:], AF.Exp, accum_out=s1[:])
                edd = apool.tile([1, 1], F32, name="edd")
                nc.scalar.activation(edd[:], sdd[:], AF.Exp)
                stot = apool.tile([1, 1], F32, name="stot")
                nc.vector.tensor_add(stot[:], s1[:], edd[:])
                rs = apool.tile([1, 1], F32, name="rs")
                nc.vector.reciprocal(rs[:], stot[:])
                nc.vector.tensor_scalar_mul(att[:], att[:], rs[:])
                # SP HWDGE: att(b-2) is long ready when SP reaches this
                # trigger, so no sequencer stall (SWDGE's software ring is
                # slow and was adding ~3us of output tail)
                nc.sync.dma_start(out[b : b + 1, 0:S], att[:])

            def do_scores(b, o1T):
                ssc, sdd = score_mms(b, o1T)
                score_post(b, ssc, sdd)

            prevs = []
            for b in range(BL):
                x1t = x1pool.tile([128, KV * S], F16, tag="x1t", name="x1t")
                x1_v = x1t[:].rearrange("p (k s) -> p k s", k=KV)
                x2t = x2pool.tile([128, KV * S], F8, tag="x2t", name="x2t")
                x2_v = x2t[:].rearrange("p (k s) -> p k s", k=KV)
                if b == 0:
                    # startup schedule on one fast queue: weights and the
                    # first example's X interleaved so the PE starts on k0-1
                    # after ~2 small transfers and then chases the stream
                    x1src = x1d[b].rearrange("p (k s) -> p k s", k=KV)
                    nc.sync.dma_start(w16[:, 0 : 2 * H], w16d[:, 0 : 2 * H])
                    nc.sync.dma_start(x1_v[:, 0:2, :], x1src[:, 0:2, :])
                    nc.sync.dma_start(w16[:, 2 * H :], w16d[:, 2 * H :])
                    nc.sync.dma_start(x1_v[:, 2:4, :], x1src[:, 2:4, :])
                    nc.sync.dma_start(x1_v[:, 4:6, :], x1src[:, 4:6, :])
                    nc.sync.dma_start(w8[:], w8d)
                    # x2 rides the second (scalar) queue in parallel: the
                    # sync queue keeps a pure weights+x1 stream
                    nc.scalar.dma_start(mlpb_sb[:], mlpb_d)
                    nc.scalar.dma_start(mlpb64_sb[:], mlpb64_d)
                    nc.scalar.dma_start(x2t[:], x2d[b])
                else:
                    nc.sync.dma_start(x1t[:], x1d[b])
                    nc.scalar.dma_start(x2t[:], x2d[b])
                if b == 1:
                    _late_const_dmas()

                o1T = o1pool.tile([128, MH * S], F16)
                pss1 = [
                    mm1_ps.tile([128, S], F32, tag=f"p1{m}", name=f"p1{m}")
                    for m in range(MH)
                ]
                for k in range(KV):
                    for m in range(MH):
                        nc.tensor.matmul(
                            pss1[m][:],
                            w16_v[:, k, m * 128 : (m + 1) * 128],
                            x1_v[:, k, :],
                            start=(k == 0),
                            stop=(k == KV - 1),
                        )

                pss2 = [
                    mm2_ps.tile([128, S], F32, tag=f"p2{m}", name=f"p2{m}")
                    for m in range(MH)
                ]
                for j in range(KP):
                    for m in range(MH):
                        nc.tensor.matmul(
                            pss2[m][:],
                            w8_v[:, 2 * j : 2 * j + 2, m * 128 : (m + 1) * 128],
                            x2_v[:, 2 * j : 2 * j + 2, :],
                            start=(j == 0),
                            stop=(j == KP - 1),
                            perf_mode=DR,
                        )
                for m in range(MH):
                    nc.scalar.activation(
                        o1T[:, m * S : (m + 1) * S],
                        pss1[m][:],
                        AF.Relu,
                        bias=mlpb_sb[:, m : m + 1],
                        accum_out=docs_all[:, b * 4 + m : b * 4 + m + 1],
                    )
                for m in range(MH):
                    # relu(p/64 + b) * 64 = max(p + 64b, 0); the 64 folds into
                    # the dscs scaling below. Only the doc sum is consumed.
                    # On the last example m0 goes to ACT so the two tail
                    # evictions run in parallel across engines.
                    if b == BL - 1 and m == 0:
                        o2scr = o2pool.tile([128, S], F16, tag="o2scr",
                                            name="o2scr")
                        nc.scalar.activation(
                            o2scr[:],
                            pss2[m][:],
                            AF.Relu,
                            bias=mlpb_sb[:, m : m + 1],
                            scale=1.0 / WSCALE,
                            accum_out=docs_all[:, b * 4 + 2 : b * 4 + 3],
                        )
                        # ACT path wrote the true-scale sum; scale it as o1
                        nc.vector.tensor_scalar_mul(
                            dscs[:, b * 4 + 2 : b * 4 + 3],
                            docs_all[:, b * 4 + 2 : b * 4 + 3], 1.0 / S)
                        continue
                    o2scr = o2pool.tile([128, S], F16, tag="o2scr", name="o2scr")
                    nc.vector.tensor_scalar(
                        o2scr[:],
                        pss2[m][:],
                        mlpb64_sb[:, m : m + 1],
                        0.0,
                        mybir.AluOpType.add,
                        mybir.AluOpType.max,
                    )
                    nc.vector.tensor_reduce(
                        docs_all[:, b * 4 + 2 + m : b * 4 + 3 + m],
                        o2scr[:],
                        mybir.AxisListType.X,
                        mybir.AluOpType.add,
                    )
                    nc.vector.tensor_scalar_mul(
                        dscs[:, b * 4 + 2 + m : b * 4 + 3 + m],
                        docs_all[:, b * 4 + 2 + m : b * 4 + 3 + m],
                        1.0 / (S * WSCALE))
                nc.vector.tensor_scalar_mul(
                    dscs[:, b * 4 : b * 4 + 2],
                    docs_all[:, b * 4 : b * 4 + 2], 1.0 / S)
                # scores run two examples behind: their inputs have been
                # ready for a full example, so the PE never stalls on them
                prevs.append((b, o1T))
                if len(prevs) > 2:
                    do_scores(*prevs.pop(0))
            # ---- tail: final scores interleaved with the head ----
            # head columns 0..BL-2 only need the first BL-1 doc vectors, so
            # they run before ex BL-1's eviction chain completes; their ph
            # psums reuse the (long-evicted) o1 psum banks.
            hpool = mctx.enter_context(tc.tile_pool(name="head", bufs=1))
            docs_v = dscs[:].rearrange("p (b k) -> p k b", k=4)
            fdw_v = fdw_sb[:].rearrange("p (k h) -> p k h", k=4)
            h16 = hpool.tile([128, MH * BL], F16)

            def head_cols(lo, hi):
                for m in range(MH):
                    ph = mm1_ps.tile([128, hi - lo], F32, tag=f"p1{m}",
                                     name=f"ph{m}")
                    for kc in range(4):
                        nc.tensor.matmul(
                            ph[:],
                            fdw_v[:, kc, m * 128 : (m + 1) * 128],
                            docs_v[:, kc, lo:hi],
                            start=(kc == 0),
                            stop=(kc == 3),
                        )
                    nc.scalar.activation(
                        h16[:, m * BL + lo : m * BL + hi],
                        ph[:],
                        AF.Relu,
                        bias=fdb_sb[:, m : m + 1],
                    )

            do_scores(*prevs.pop(0))
            head_cols(0, BL - 1)
            do_scores(*prevs.pop(0))
            head_cols(BL - 1, BL)

            po = dd_ps.tile([1, BL], F32, tag="sdd", name="po")
            for m in range(MH):
                nc.tensor.matmul(
                    po[:],
                    ffw_sb[:, m : m + 1],
                    h16[:, m * BL : (m + 1) * BL],
                    start=(m == 0),
                    stop=(m == MH - 1),
                )
            # sigmoid(x) = 1/(1+exp(-x)) — stays in the Exp table set
            sig_row = hpool.tile([1, BL], F32)
            nc.scalar.activation(sig_row[:], po[:], AF.Exp,
                                 bias=nffb_sb[0:1, 0:1], scale=-1.0)
            nc.vector.tensor_scalar_add(sig_row[:], sig_row[:], 1.0)
            nc.vector.reciprocal(sig_row[:], sig_row[:])

            # final output column: out[:, S] = sigmoid head values
            nc.sync.dma_start(
                out[:, S : S + 1],
                sig_row[0:1, :].rearrange("o (b s) -> o b s", b=BL),
            )


_NC_CACHE = None


def _get_nc():
    global _NC_CACHE
    if _NC_CACHE is None:
        nc = bacc.Bacc("TRN2", target_bir_lowering=False, debug=False,
                       num_devices=NCORES)
        with tile.TileContext(nc) as tc:
            _build_kernel(tc)
        nc.compile()
        _NC_CACHE = nc
    return _NC_CACHE


def kernel(output_1, output_2, mlp_w, mlp_b, fd_w, fd_b, ff_w, ff_b):
    output_1 = np.asarray(output_1, dtype=np.float32)
    output_2 = np.asarray(output_2, dtype=np.float32)
    mlp_w = np.asarray(mlp_w, dtype=np.float32)
    mlp_b = np.asarray(mlp_b, dtype=np.float32)
    fd_w = np.asarray(fd_w, dtype=np.float32)
    fd_b = np.asarray(fd_b, dtype=np.float32)
    ff_w = np.asarray(ff_w, dtype=np.float32)
    ff_b = np.asarray(ff_b, dtype=np.float32)

    # shard over batch; [*, S, V] -> [*, p, k, S] with V = k*128 + p
    def to_pks(x):
        return np.ascontiguousarray(
            x.reshape(NCORES, BL, S, KV, 128).transpose(0, 1, 4, 3, 2))

    x1 = to_pks(output_1).astype(np.float16).reshape(NCORES, BL, 128, KV * S)
    x2 = to_pks(output_2).astype(E4NP).reshape(NCORES, BL, 128, KV * S)

    wt = np.ascontiguousarray(
        mlp_w.T.reshape(KV, 128, H).transpose(1, 0, 2))        # [p, k, H]
    w16 = wt.astype(np.float16).reshape(128, KV * H)
    w8 = (wt * WSCALE).astype(E4NP).reshape(128, KV * H)

    mlpb = np.ascontiguousarray(mlp_b.reshape(MH, 128).T)      # [128, MH]
    mlpb64 = np.ascontiguousarray(mlpb * WSCALE)
    fdw = np.ascontiguousarray(
        fd_w.T.reshape(4, 128, H).transpose(1, 0, 2)
    ).reshape(128, 4 * H).astype(np.float16)
    fdb = np.ascontiguousarray(fd_b.reshape(MH, 128).T)
    ffw = np.ascontiguousarray(
        ff_w.reshape(MH, 128).T).astype(np.float16)            # [128, MH]
    nffb = np.ascontiguousarray(-ff_b.reshape(1, 1))

    in_maps = [
        dict(x1=x1[c], x2=x2[c], w16=w16, w8=w8, mlpb=mlpb, mlpb64=mlpb64,
             fdw=fdw, fdb=fdb, ffw=ffw, nffb=nffb)
        for c in range(NCORES)
    ]
    global _LAST_IN_MAPS
    _LAST_IN_MAPS = in_maps
    nc = _get_nc()
    res = bass_utils.run_bass_kernel_spmd(nc, in_maps, core_ids=list(range(NCORES)))
    att = np.concatenate([res.results[c]["out"] for c in range(NCORES)], axis=0)
    return np.ascontiguousarray(att.T)  # [S+1, B]


# revision 56
# speedup vs baseline: 1.2405x; 1.0353x over previous
"""Bass/Trainium2 kernel for nn_Bert_coss (8-core data-parallel over batch).

Computation (per example):
  o1 = relu(X1 @ W.T + b)            [S, H]
  o2 = relu(X2 @ W.T + b)            [S, H]
  o1_doc, o2_doc = mean over S       [H]
  out = sigmoid(relu(concat(o1_doc, o2_doc) @ fd_w.T + fd_b) @ ff_w.T + ff_b)
  scores[s] = o1e[s] . o2_doc   (o1e = o1 ++ o1_doc row), s in 0..S
  att = softmax(scores); output rows 0..S-1 = att[0:S], row S = out.

The reference's full [S+1,S+1] co-attention einsum is only consumed through
its last column, so only S+1 dot products against o2_doc are needed.

Precision/throughput split (calibrated on-HW: fp16 matmul ~507ns per
K256x128x512 block, fp8e4 DoubleRow ~292ns):
 - o1 path feeds the softmax scores directly and needs full precision:
   fp16 matmuls (12 x 512-row instructions per example).
 - o2 is only consumed through its S-mean doc vector: per-entry fp8 noise
   averages out over 512 positions and its shared component cancels in the
   softmax, so one fp8(E4M3) DoubleRow product (6 instructions, W pre-scaled
   by 64 into e4m3's range) suffices. Validated end-to-end ~1.5e-3 rel err
   vs the 2e-2 gate.
DMA drops to 3 bytes per (X1,X2) element pair; PE drops 25% vs all-fp16.

Evictions are split across engines to keep each under the DMA-bound
critical path: o1 relu+bias on ACT (accum_out = doc sums), o2 relu on DVE
as max(psum + 64b, 0) followed by an add-reduce (the 64 folds into the
doc-scale constants). Score softmax normalization runs on GpSimd.
"""

import sys

for _p in ("/opt/trn_rl_repo",):
    if _p not in sys.path:
        sys.path.append(_p)

import numpy as np
import ml_dtypes
from contextlib import ExitStack

import concourse.bass as bass
import concourse.tile as tile
from concourse import bacc, mybir
from concourse import bass_utils

B, S, V, H = 64, 512, 768, 256
NCORES = 8
BL = B // NCORES        # examples per core
KV = V // 128           # contraction chunks for the mlp matmul
KP = KV // 2            # DoubleRow k-pairs
MH = H // 128           # output-partition chunks of H
WSCALE = 64.0           # W pre-scale so fp8 e4m3 covers its range

F32 = mybir.dt.float32
F16 = mybir.dt.float16
F8 = mybir.dt.float8e4
AF = mybir.ActivationFunctionType
DR = mybir.MatmulPerfMode.DoubleRow
E4NP = ml_dtypes.float8_e4m3


def _build_kernel(tc):
    nc = tc.nc
    x1d = nc.dram_tensor("x1", [BL, 128, KV * S], F16,
                         kind="ExternalInput").ap()
    x2d = nc.dram_tensor("x2", [BL, 128, KV * S], F8,
                         kind="ExternalInput").ap()
    w16d = nc.dram_tensor("w16", [128, KV * H], F16, kind="ExternalInput").ap()
    w8d = nc.dram_tensor("w8", [128, KV * H], F8, kind="ExternalInput").ap()
    mlpb_d = nc.dram_tensor("mlpb", [128, MH], F32, kind="ExternalInput").ap()
    mlpb64_d = nc.dram_tensor("mlpb64", [128, MH], F32,
                              kind="ExternalInput").ap()
    fdw_d = nc.dram_tensor("fdw", [128, 4 * H], F16, kind="ExternalInput").ap()
    fdb_d = nc.dram_tensor("fdb", [128, MH], F32, kind="ExternalInput").ap()
    ffw_d = nc.dram_tensor("ffw", [128, MH], F16, kind="ExternalInput").ap()
    nffb_d = nc.dram_tensor("nffb", [1, 1], F32, kind="ExternalInput").ap()
    out = nc.dram_tensor("out", [BL, S + 1], F32, kind="ExternalOutput").ap()

    with ExitStack() as ctx:
        const = ctx.enter_context(tc.tile_pool(name="const", bufs=1))

        # PE-critical params ride the fast sync queue, interleaved with the
        # first example's X chunks (issued below) so the PE starts ~9.8us
        w16 = const.tile([128, KV * H], F16)
        w16_v = w16[:].rearrange("p (k h) -> p k h", k=KV)
        w8 = const.tile([128, KV * H], F8)
        w8_v = w8[:].rearrange("p (k h) -> p k h", k=KV)
        mlpb_sb = const.tile([128, MH], F32)
        mlpb64_sb = const.tile([128, MH], F32)
        fdw_sb = const.tile([128, 4 * H], F16)
        fdb_sb = const.tile([128, MH], F32)
        ffw_sb = const.tile([128, MH], F16)
        nffb_sb = const.tile([1, 1], F32)
        # dummy Exp so the ACT table set loads during the DMA ramp instead of
        # on the end-of-kernel critical path (scale=0 -> input value unused)
        expwarm = const.tile([1, 1], F32)
        nc.scalar.activation(expwarm[:], expwarm[:], AF.Exp, scale=0.0)
        # scratch operands for PE clock-warming matmuls (values irrelevant,
        # the framework just requires a write before reads)
        pewarm = const.tile([128, S], F16)
        nc.vector.memset(pewarm[:], 0.0)


        def _late_const_dmas():
            # parameters only needed by the end-of-kernel head
            nc.scalar.dma_start(fdw_sb[:], fdw_d)
            nc.scalar.dma_start(fdb_sb[:], fdb_d)
            nc.scalar.dma_start(ffw_sb[:], ffw_d)
            nc.scalar.dma_start(nffb_sb[:], nffb_d)

        # doc-vector raw sums; column b*4 + c, c in (o1m0, o1m1, o2m0, o2m1)
        # o1 columns hold 512*o1_doc; o2 columns hold 512*64*o2_doc
        docs_all = const.tile([128, 4 * BL], F32)
        # true-scale doc vectors in fp16 (score matvec lhsT + head rhs)
        dscs = const.tile([128, 4 * BL], F16)

        with ExitStack() as mctx:
            x1pool = mctx.enter_context(tc.tile_pool(name="x1", bufs=4))
            x2pool = mctx.enter_context(tc.tile_pool(name="x2", bufs=4))
            o1pool = mctx.enter_context(tc.tile_pool(name="o1", bufs=3))
            o2pool = mctx.enter_context(tc.tile_pool(name="o2", bufs=2))
            apool = mctx.enter_context(tc.tile_pool(name="att", bufs=3))
            mm1_ps = mctx.enter_context(tc.tile_pool(name="mm1", bufs=2, space="PSUM"))
            mm2_ps = mctx.enter_context(tc.tile_pool(name="mm2", bufs=1, space="PSUM"))
            sc_ps = mctx.enter_context(tc.tile_pool(name="scps", bufs=1, space="PSUM"))
            dd_ps = mctx.enter_context(tc.tile_pool(name="ddps", bufs=1, space="PSUM"))

            def score_mms(b, o1T):
                ssc = sc_ps.tile([1, S], F32, tag="ssc", name="ssc")
                for hk in range(MH):
                    nc.tensor.matmul(
                        ssc[:],
                        dscs[:, b * 4 + 2 + hk : b * 4 + 3 + hk],
                        o1T[:, hk * S : (hk + 1) * S],
                        start=(hk == 0),
                        stop=(hk == MH - 1),
                    )
                sdd = dd_ps.tile([1, 1], F32, tag="sdd", name="sdd")
                for hk in range(MH):
                    nc.tensor.matmul(
                        sdd[:],
                        dscs[:, b * 4 + 2 + hk : b * 4 + 3 + hk],
                        dscs[:, b * 4 + hk : b * 4 + hk + 1],
                        start=(hk == 0),
                        stop=(hk == MH - 1),
                    )
                return ssc, sdd

            def score_post(b, ssc, sdd):
                # softmax on partition 0, straight from PSUM; no max-
                # subtraction (scores are O(25), far inside fp32 exp range)
                att = apool.tile([1, S], F32)
                s1 = apool.tile([1, 1], F32, name="s1")
                nc.scalar.activation(att[:], ssc[:], AF.Exp, accum_out=s1[:])
                edd = apool.tile([1, 1], F32, name="edd")
                nc.scalar.activation(edd[:], sdd[:], AF.Exp)
                stot = apool.tile([1, 1], F32, name="stot")
                nc.vector.tensor_add(stot[:], s1[:], edd[:])
                rs = apool.tile([1, 1], F32, name="rs")
                nc.vector.reciprocal(rs[:], stot[:])
                nc.vector.tensor_scalar_mul(att[:], att[:], rs[:])
                # SP HWDGE: att(b-2) is long ready when SP reaches this
                # trigger, so no sequencer stall (SWDGE's software ring is
                # slow and was adding ~3us of output tail)
                nc.sync.dma_start(out[b : b + 1, 0:S], att[:])

            def do_scores(b, o1T):
                ssc, sdd = score_mms(b, o1T)
                score_post(b, ssc, sdd)

            # PE clock warm-up, sized to end as the first input chunk lands:
            # the power manager grants full clock ~4.5us after activity
            # starts, so 8 dummy matmuls during the otherwise-dead DMA ramp
            # put the PE at full speed for the first real matmul. (Oversized
            # warm-ups, tail keep-alives and mid-stream fillers all measured
            # WORSE — the fake work must stay inside the dead window.)
            kawarm = sc_ps.tile([1, S], F32, tag="ssc", name="kawarm")
            for _ in range(8):
                nc.tensor.matmul(
                    kawarm[:], pewarm[:, 0:1], pewarm[:],
                    start=True, stop=True,
                )

            prevs = []
            for b in range(BL):
                x1t = x1pool.tile([128, KV * S], F16, tag="x1t", name="x1t")
                x1_v = x1t[:].rearrange("p (k s) -> p k s", k=KV)
                x2t = x2pool.tile([128, KV * S], F8, tag="x2t", name="x2t")
                x2_v = x2t[:].rearrange("p (k s) -> p k s", k=KV)
                if b == 0:
                    # startup schedule on one fast queue: weights and the
                    # first example's X interleaved so the PE starts on k0-1
                    # after ~2 small transfers and then chases the stream
                    x1src = x1d[b].rearrange("p (k s) -> p k s", k=KV)
                    nc.sync.dma_start(w16[:], w16d[:, :])
                    nc.sync.dma_start(x1_v[:, 0:2, :], x1src[:, 0:2, :])
                    nc.sync.dma_start(x1_v[:, 2:4, :], x1src[:, 2:4, :])
                    nc.sync.dma_start(x1_v[:, 4:6, :], x1src[:, 4:6, :])
                    nc.sync.dma_start(w8[:], w8d)
                    # x2 rides the second (scalar) queue in parallel: the
                    # sync queue keeps a pure weights+x1 stream
                    nc.scalar.dma_start(mlpb_sb[:], mlpb_d)
                    nc.scalar.dma_start(mlpb64_sb[:], mlpb64_d)
                    nc.scalar.dma_start(x2t[:], x2d[b])
                else:
                    nc.sync.dma_start(x1t[:], x1d[b])
                    nc.scalar.dma_start(x2t[:], x2d[b])
                if b == 1:
                    _late_const_dmas()

                o1T = o1pool.tile([128, MH * S], F16)
                pss1 = [
                    mm1_ps.tile([128, S], F32, tag=f"p1{m}", name=f"p1{m}")
                    for m in range(MH)
                ]
                for k in range(KV):
                    for m in range(MH):
                        nc.tensor.matmul(
                            pss1[m][:],
                            w16_v[:, k, m * 128 : (m + 1) * 128],
                            x1_v[:, k, :],
                            start=(k == 0),
                            stop=(k == KV - 1),
                        )

                pss2 = [
                    mm2_ps.tile([128, S], F32, tag=f"p2{m}", name=f"p2{m}")
                    for m in range(MH)
                ]
                for j in range(KP):
                    for m in range(MH):
                        nc.tensor.matmul(
                            pss2[m][:],
                            w8_v[:, 2 * j : 2 * j + 2, m * 128 : (m + 1) * 128],
                            x2_v[:, 2 * j : 2 * j + 2, :],
                            start=(j == 0),
                            stop=(j == KP - 1),
                            perf_mode=DR,
                        )
                for m in range(MH):
                    nc.scalar.activation(
                        o1T[:, m * S : (m + 1) * S],
                        pss1[m][:],
                        AF.Relu,
                        bias=mlpb_sb[:, m : m + 1],
                        accum_out=docs_all[:, b * 4 + m : b * 4 + m + 1],
                    )
                for m in range(MH):
                    # relu(p/64 + b) * 64 = max(p + 64b, 0); the 64 folds into
                    # the dscs scaling below. Only the doc sum is consumed.
                    # On the last example m0 goes to ACT so the two tail
                    # evictions run in parallel across engines.
                    if b == BL - 1 and m == 0:
                        o2scr = o2pool.tile([128, S], F16, tag="o2scr",
                                            name="o2scr")
                        nc.scalar.activation(
                            o2scr[:],
                            pss2[m][:],
                            AF.Relu,
                            bias=mlpb_sb[:, m : m + 1],
                            scale=1.0 / WSCALE,
                            accum_out=docs_all[:, b * 4 + 2 : b * 4 + 3],
                        )
                        # ACT path wrote the true-scale sum; scale it as o1
                        nc.vector.tensor_scalar_mul(
                            dscs[:, b * 4 + 2 : b * 4 + 3],
                            docs_all[:, b * 4 + 2 : b * 4 + 3], 1.0 / S)
                        continue
                    o2scr = o2pool.tile([128, S], F16, tag="o2scr", name="o2scr")
                    nc.vector.tensor_scalar(
                        o2scr[:],
                        pss2[m][:],
                        mlpb64_sb[:, m : m + 1],
                        0.0,
                        mybir.AluOpType.add,
                        mybir.AluOpType.max,
                    )
                    nc.vector.tensor_reduce(
                        docs_all[:, b * 4 + 2 + m : b * 4 + 3 + m],
                        o2scr[:],
                        mybir.AxisListType.X,
                        mybir.AluOpType.add,
                    )
                    nc.vector.tensor_scalar_mul(
                        dscs[:, b * 4 + 2 + m : b * 4 + 3 + m],
                        docs_all[:, b * 4 + 2 + m : b * 4 + 3 + m],
                        1.0 / (S * WSCALE))
                nc.vector.tensor_scalar_mul(
                    dscs[:, b * 4 : b * 4 + 2],
                    docs_all[:, b * 4 : b * 4 + 2], 1.0 / S)
                # scores run two examples behind: their inputs have been
                # ready for a full example, so the PE never stalls on them
                prevs.append((b, o1T))
                if len(prevs) > 2:
                    do_scores(*prevs.pop(0))
            # ---- tail: final scores interleaved with the head ----
            # head columns 0..BL-2 only need the first BL-1 doc vectors, so
            # they run before ex BL-1's eviction chain completes; their ph
            # psums reuse the (long-evicted) o1 psum banks.
            hpool = mctx.enter_context(tc.tile_pool(name="head", bufs=1))
            docs_v = dscs[:].rearrange("p (b k) -> p k b", k=4)
            fdw_v = fdw_sb[:].rearrange("p (k h) -> p k h", k=4)
            h16 = hpool.tile([128, MH * BL], F16)

            def head_cols(lo, hi):
                for m in range(MH):
                    ph = mm1_ps.tile([128, hi - lo], F32, tag=f"p1{m}",
                                     name=f"ph{m}")
                    for kc in range(4):
                        nc.tensor.matmul(
                            ph[:],
                            fdw_v[:, kc, m * 128 : (m + 1) * 128],
                            docs_v[:, kc, lo:hi],
                            start=(kc == 0),
                            stop=(kc == 3),
                        )
                    nc.scalar.activation(
                        h16[:, m * BL + lo : m * BL + hi],
                        ph[:],
                        AF.Relu,
                        bias=fdb_sb[:, m : m + 1],
                    )

            do_scores(*prevs.pop(0))
            head_cols(0, BL - 1)
            do_scores(*prevs.pop(0))
            head_cols(BL - 1, BL)

            po = dd_ps.tile([1, BL], F32, tag="sdd", name="po")
            for m in range(MH):
                nc.tensor.matmul(
                    po[:],
                    ffw_sb[:, m : m + 1],
                    h16[:, m * BL : (m + 1) * BL],
                    start=(m == 0),
                    stop=(m == MH - 1),
                )
            # sigmoid(x) = 1/(1+exp(-x)) — stays in the Exp table set
            sig_row = hpool.tile([1, BL], F32)
            nc.scalar.activation(sig_row[:], po[:], AF.Exp,
                                 bias=nffb_sb[0:1, 0:1], scale=-1.0)
            nc.vector.tensor_scalar_add(sig_row[:], sig_row[:], 1.0)
            nc.vector.reciprocal(sig_row[:], sig_row[:])

            # final output column: out[:, S] = sigmoid head values
            nc.sync.dma_start(
                out[:, S : S + 1],
                sig_row[0:1, :].rearrange("o (b s) -> o b s", b=BL),
            )


_NC_CACHE = None


def _get_nc():
    global _NC_CACHE
    if _NC_CACHE is None:
        nc = bacc.Bacc("TRN2", target_bir_lowering=False, debug=False,
                       num_devices=NCORES)
        with tile.TileContext(nc) as tc:
            _build_kernel(tc)
        nc.compile()
        _NC_CACHE = nc
    return _NC_CACHE


def kernel(output_1, output_2, mlp_w, mlp_b, fd_w, fd_b, ff_w, ff_b):
    output_1 = np.asarray(output_1, dtype=np.float32)
    output_2 = np.asarray(output_2, dtype=np.float32)
    mlp_w = np.asarray(mlp_w, dtype=np.float32)
    mlp_b = np.asarray(mlp_b, dtype=np.float32)
    fd_w = np.asarray(fd_w, dtype=np.float32)
    fd_b = np.asarray(fd_b, dtype=np.float32)
    ff_w = np.asarray(ff_w, dtype=np.float32)
    ff_b = np.asarray(ff_b, dtype=np.float32)

    # shard over batch; [*, S, V] -> [*, p, k, S] with V = k*128 + p
    def to_pks(x):
        return np.ascontiguousarray(
            x.reshape(NCORES, BL, S, KV, 128).transpose(0, 1, 4, 3, 2))

    x1 = to_pks(output_1).astype(np.float16).reshape(NCORES, BL, 128, KV * S)
    x2 = to_pks(output_2).astype(E4NP).reshape(NCORES, BL, 128, KV * S)

    wt = np.ascontiguousarray(
        mlp_w.T.reshape(KV, 128, H).transpose(1, 0, 2))        # [p, k, H]
    w16 = wt.astype(np.float16).reshape(128, KV * H)
    w8 = (wt * WSCALE).astype(E4NP).reshape(128, KV * H)

    mlpb = np.ascontiguousarray(mlp_b.reshape(MH, 128).T)      # [128, MH]
    mlpb64 = np.ascontiguousarray(mlpb * WSCALE)
    fdw = np.ascontiguousarray(
        fd_w.T.reshape(4, 128, H).transpose(1, 0, 2)
    ).reshape(128, 4 * H).astype(np.float16)
    fdb = np.ascontiguousarray(fd_b.reshape(MH, 128).T)
    ffw = np.ascontiguousarray(
        ff_w.reshape(MH, 128).T).astype(np.float16)            # [128, MH]
    nffb = np.ascontiguousarray(-ff_b.reshape(1, 1))

    in_maps = [
        dict(x1=x1[c], x2=x2[c], w16=w16, w8=w8, mlpb=mlpb, mlpb64=mlpb64,
             fdw=fdw, fdb=fdb, ffw=ffw, nffb=nffb)
        for c in range(NCORES)
    ]
    global _LAST_IN_MAPS
    _LAST_IN_MAPS = in_maps
    nc = _get_nc()
    res = bass_utils.run_bass_kernel_spmd(nc, in_maps, core_ids=list(range(NCORES)))
    att = np.concatenate([res.results[c]["out"] for c in range(NCORES)], axis=0)
    return np.ascontiguousarray(att.T)  # [S+1, B]


# revision 57
# speedup vs baseline: 1.2591x; 1.0151x over previous
"""Bass/Trainium2 kernel for nn_Bert_coss (8-core data-parallel over batch).

Computation (per example):
  o1 = relu(X1 @ W.T + b)            [S, H]
  o2 = relu(X2 @ W.T + b)            [S, H]
  o1_doc, o2_doc = mean over S       [H]
  out = sigmoid(relu(concat(o1_doc, o2_doc) @ fd_w.T + fd_b) @ ff_w.T + ff_b)
  scores[s] = o1e[s] . o2_doc   (o1e = o1 ++ o1_doc row), s in 0..S
  att = softmax(scores); output rows 0..S-1 = att[0:S], row S = out.

The reference's full [S+1,S+1] co-attention einsum is only consumed through
its last column, so only S+1 dot products against o2_doc are needed.

Precision/throughput split (calibrated on-HW: fp16 matmul ~507ns per
K256x128x512 block, fp8e4 DoubleRow ~292ns):
 - o1 path feeds the softmax scores directly and needs full precision:
   fp16 matmuls (12 x 512-row instructions per example).
 - o2 is only consumed through its S-mean doc vector: per-entry fp8 noise
   averages out over 512 positions and its shared component cancels in the
   softmax, so one fp8(E4M3) DoubleRow product (6 instructions, W pre-scaled
   by 64 into e4m3's range) suffices. Validated end-to-end ~1.5e-3 rel err
   vs the 2e-2 gate.
DMA drops to 3 bytes per (X1,X2) element pair; PE drops 25% vs all-fp16.

Evictions are split across engines to keep each under the DMA-bound
critical path: o1 relu+bias on ACT (accum_out = doc sums), o2 relu on DVE
as max(psum + 64b, 0) followed by an add-reduce (the 64 folds into the
doc-scale constants). Score softmax normalization runs on GpSimd.
"""

import sys

for _p in ("/opt/trn_rl_repo",):
    if _p not in sys.path:
        sys.path.append(_p)

import numpy as np
import ml_dtypes
from contextlib import ExitStack

import concourse.bass as bass
import concourse.tile as tile
from concourse import bacc, mybir
from concourse import bass_utils

B, S, V, H = 64, 512, 768, 256
NCORES = 8
BL = B // NCORES        # examples per core
KV = V // 128           # contraction chunks for the mlp matmul
KP = KV // 2            # DoubleRow k-pairs
MH = H // 128           # output-partition chunks of H
WSCALE = 64.0           # W pre-scale so fp8 e4m3 covers its range

F32 = mybir.dt.float32
F16 = mybir.dt.float16
F8 = mybir.dt.float8e4
AF = mybir.ActivationFunctionType
DR = mybir.MatmulPerfMode.DoubleRow
E4NP = ml_dtypes.float8_e4m3


def _build_kernel(tc):
    nc = tc.nc
    x1d = nc.dram_tensor("x1", [BL, 128, KV * S], F16,
                         kind="ExternalInput").ap()
    x2d = nc.dram_tensor("x2", [BL, 128, KV * S], F8,
                         kind="ExternalInput").ap()
    w16d = nc.dram_tensor("w16", [128, KV * H], F16, kind="ExternalInput").ap()
    w8d = nc.dram_tensor("w8", [128, KV * H], F8, kind="ExternalInput").ap()
    mlpb_d = nc.dram_tensor("mlpb", [128, MH], F32, kind="ExternalInput").ap()
    mlpb64_d = nc.dram_tensor("mlpb64", [128, MH], F32,
                              kind="ExternalInput").ap()
    fdw_d = nc.dram_tensor("fdw", [128, 4 * H], F16, kind="ExternalInput").ap()
    fdb_d = nc.dram_tensor("fdb", [128, MH], F32, kind="ExternalInput").ap()
    ffw_d = nc.dram_tensor("ffw", [128, MH], F16, kind="ExternalInput").ap()
    nffb_d = nc.dram_tensor("nffb", [1, 1], F32, kind="ExternalInput").ap()
    out = nc.dram_tensor("out", [BL, S + 1], F32, kind="ExternalOutput").ap()

    with ExitStack() as ctx:
        const = ctx.enter_context(tc.tile_pool(name="const", bufs=1))

        # PE-critical params ride the fast sync queue, interleaved with the
        # first example's X chunks (issued below) so the PE starts ~9.8us
        w16 = const.tile([128, KV * H], F16)
        w16_v = w16[:].rearrange("p (k h) -> p k h", k=KV)
        w8 = const.tile([128, KV * H], F8)
        w8_v = w8[:].rearrange("p (k h) -> p k h", k=KV)
        mlpb_sb = const.tile([128, MH], F32)
        mlpb64_sb = const.tile([128, MH], F32)
        fdw_sb = const.tile([128, 4 * H], F16)
        fdb_sb = const.tile([128, MH], F32)
        ffw_sb = const.tile([128, MH], F16)
        nffb_sb = const.tile([1, 1], F32)
        # dummy Exp so the ACT table set loads during the DMA ramp instead of
        # on the end-of-kernel critical path (scale=0 -> input value unused)
        expwarm = const.tile([1, 1], F32)
        nc.scalar.activation(expwarm[:], expwarm[:], AF.Exp, scale=0.0)
        # scratch operands for PE clock-warming matmuls (values irrelevant,
        # the framework just requires a write before reads)
        pewarm = const.tile([128, S], F16)
        nc.vector.memset(pewarm[:], 0.0)


        def _late_const_dmas():
            # parameters only needed by the end-of-kernel head
            nc.scalar.dma_start(fdw_sb[:], fdw_d)
            nc.scalar.dma_start(fdb_sb[:], fdb_d)
            nc.scalar.dma_start(ffw_sb[:], ffw_d)
            nc.scalar.dma_start(nffb_sb[:], nffb_d)

        # doc-vector raw sums; column b*4 + c, c in (o1m0, o1m1, o2m0, o2m1)
        # o1 columns hold 512*o1_doc; o2 columns hold 512*64*o2_doc
        docs_all = const.tile([128, 4 * BL], F32)
        # true-scale doc vectors in fp16 (score matvec lhsT + head rhs)
        dscs = const.tile([128, 4 * BL], F16)

        with ExitStack() as mctx:
            x1pool = mctx.enter_context(tc.tile_pool(name="x1", bufs=4))
            x2pool = mctx.enter_context(tc.tile_pool(name="x2", bufs=4))
            o1pool = mctx.enter_context(tc.tile_pool(name="o1", bufs=3))
            o2pool = mctx.enter_context(tc.tile_pool(name="o2", bufs=2))
            apool = mctx.enter_context(tc.tile_pool(name="att", bufs=3))
            mm1_ps = mctx.enter_context(tc.tile_pool(name="mm1", bufs=2, space="PSUM"))
            mm2_ps = mctx.enter_context(tc.tile_pool(name="mm2", bufs=1, space="PSUM"))
            sc_ps = mctx.enter_context(tc.tile_pool(name="scps", bufs=1, space="PSUM"))
            dd_ps = mctx.enter_context(tc.tile_pool(name="ddps", bufs=1, space="PSUM"))

            def score_mms(b, o1T):
                ssc = sc_ps.tile([1, S], F32, tag="ssc", name="ssc")
                for hk in range(MH):
                    nc.tensor.matmul(
                        ssc[:],
                        dscs[:, b * 4 + 2 + hk : b * 4 + 3 + hk],
                        o1T[:, hk * S : (hk + 1) * S],
                        start=(hk == 0),
                        stop=(hk == MH - 1),
                    )
                sdd = dd_ps.tile([1, 1], F32, tag="sdd", name="sdd")
                for hk in range(MH):
                    nc.tensor.matmul(
                        sdd[:],
                        dscs[:, b * 4 + 2 + hk : b * 4 + 3 + hk],
                        dscs[:, b * 4 + hk : b * 4 + hk + 1],
                        start=(hk == 0),
                        stop=(hk == MH - 1),
                    )
                return ssc, sdd

            def score_post(b, ssc, sdd):
                # softmax on partition 0, straight from PSUM; no max-
                # subtraction (scores are O(25), far inside fp32 exp range)
                att = apool.tile([1, S], F32)
                s1 = apool.tile([1, 1], F32, name="s1")
                nc.scalar.activation(att[:], ssc[:], AF.Exp, accum_out=s1[:])
                edd = apool.tile([1, 1], F32, name="edd")
                nc.scalar.activation(edd[:], sdd[:], AF.Exp)
                stot = apool.tile([1, 1], F32, name="stot")
                nc.vector.tensor_add(stot[:], s1[:], edd[:])
                rs = apool.tile([1, 1], F32, name="rs")
                nc.vector.reciprocal(rs[:], stot[:])
                nc.vector.tensor_scalar_mul(att[:], att[:], rs[:])
                # SP HWDGE: att(b-2) is long ready when SP reaches this
                # trigger, so no sequencer stall (SWDGE's software ring is
                # slow and was adding ~3us of output tail)
                nc.sync.dma_start(out[b : b + 1, 0:S], att[:])

            def do_scores(b, o1T):
                ssc, sdd = score_mms(b, o1T)
                score_post(b, ssc, sdd)

            # PE clock warm-up, sized to end as the first input chunk lands:
            # the power manager grants full clock ~4.5us after activity
            # starts, so 8 dummy matmuls during the otherwise-dead DMA ramp
            # put the PE at full speed for the first real matmul. (Oversized
            # warm-ups, tail keep-alives and mid-stream fillers all measured
            # WORSE — the fake work must stay inside the dead window.)
            kawarm = sc_ps.tile([1, S], F32, tag="ssc", name="kawarm")
            for _ in range(8):
                nc.tensor.matmul(
                    kawarm[:], pewarm[:, 0:1], pewarm[:],
                    start=True, stop=True,
                )

            prevs = []
            for b in range(BL):
                x1t = x1pool.tile([128, KV * S], F16, tag="x1t", name="x1t")
                x1_v = x1t[:].rearrange("p (k s) -> p k s", k=KV)
                x2t = x2pool.tile([128, KV * S], F8, tag="x2t", name="x2t")
                x2_v = x2t[:].rearrange("p (k s) -> p k s", k=KV)
                if b == 0:
                    # startup schedule on one fast queue: weights and the
                    # first example's X interleaved so the PE starts on k0-1
                    # after ~2 small transfers and then chases the stream
                    x1src = x1d[b].rearrange("p (k s) -> p k s", k=KV)
                    nc.sync.dma_start(w16[:], w16d[:, :])
                    nc.sync.dma_start(x1_v[:, 0:2, :], x1src[:, 0:2, :])
                    nc.sync.dma_start(x1_v[:, 2:4, :], x1src[:, 2:4, :])
                    nc.sync.dma_start(x1_v[:, 4:6, :], x1src[:, 4:6, :])
                    nc.sync.dma_start(w8[:], w8d)
                    # x2 rides the second (scalar) queue in parallel: the
                    # sync queue keeps a pure weights+x1 stream. Its trigger
                    # goes first — each trigger costs ~600ns of sequencer
                    # time and the biases are not needed until eviction.
                    nc.scalar.dma_start(x2t[:], x2d[b])
                    nc.scalar.dma_start(mlpb_sb[:], mlpb_d)
                    nc.scalar.dma_start(mlpb64_sb[:], mlpb64_d)
                elif b <= 2:
                    # chase region: split x1 so the o1 matmuls start on the
                    # first half while the second is still in flight
                    x1src = x1d[b].rearrange("p (k s) -> p k s", k=KV)
                    nc.sync.dma_start(x1_v[:, 0:3, :], x1src[:, 0:3, :])
                    nc.sync.dma_start(x1_v[:, 3:6, :], x1src[:, 3:6, :])
                    nc.scalar.dma_start(x2t[:], x2d[b])
                else:
                    nc.sync.dma_start(x1t[:], x1d[b])
                    nc.scalar.dma_start(x2t[:], x2d[b])
                if b == 1:
                    _late_const_dmas()

                o1T = o1pool.tile([128, MH * S], F16)
                pss1 = [
                    mm1_ps.tile([128, S], F32, tag=f"p1{m}", name=f"p1{m}")
                    for m in range(MH)
                ]
                for k in range(KV):
                    for m in range(MH):
                        nc.tensor.matmul(
                            pss1[m][:],
                            w16_v[:, k, m * 128 : (m + 1) * 128],
                            x1_v[:, k, :],
                            start=(k == 0),
                            stop=(k == KV - 1),
                        )

                pss2 = [
                    mm2_ps.tile([128, S], F32, tag=f"p2{m}", name=f"p2{m}")
                    for m in range(MH)
                ]
                for j in range(KP):
                    for m in range(MH):
                        nc.tensor.matmul(
                            pss2[m][:],
                            w8_v[:, 2 * j : 2 * j + 2, m * 128 : (m + 1) * 128],
                            x2_v[:, 2 * j : 2 * j + 2, :],
                            start=(j == 0),
                            stop=(j == KP - 1),
                            perf_mode=DR,
                        )
                for m in range(MH):
                    nc.scalar.activation(
                        o1T[:, m * S : (m + 1) * S],
                        pss1[m][:],
                        AF.Relu,
                        bias=mlpb_sb[:, m : m + 1],
                        accum_out=docs_all[:, b * 4 + m : b * 4 + m + 1],
                    )
                for m in range(MH):
                    # relu(p/64 + b) * 64 = max(p + 64b, 0); the 64 folds into
                    # the dscs scaling below. Only the doc sum is consumed.
                    # On the last example m0 goes to ACT so the two tail
                    # evictions run in parallel across engines.
                    if b == BL - 1 and m == 0:
                        o2scr = o2pool.tile([128, S], F16, tag="o2scr",
                                            name="o2scr")
                        nc.scalar.activation(
                            o2scr[:],
                            pss2[m][:],
                            AF.Relu,
                            bias=mlpb_sb[:, m : m + 1],
                            scale=1.0 / WSCALE,
                            accum_out=docs_all[:, b * 4 + 2 : b * 4 + 3],
                        )
                        # ACT path wrote the true-scale sum; scale it as o1
                        nc.vector.tensor_scalar_mul(
                            dscs[:, b * 4 + 2 : b * 4 + 3],
                            docs_all[:, b * 4 + 2 : b * 4 + 3], 1.0 / S)
                        continue
                    o2scr = o2pool.tile([128, S], F16, tag="o2scr", name="o2scr")
                    nc.vector.tensor_scalar(
                        o2scr[:],
                        pss2[m][:],
                        mlpb64_sb[:, m : m + 1],
                        0.0,
                        mybir.AluOpType.add,
                        mybir.AluOpType.max,
                    )
                    nc.vector.tensor_reduce(
                        docs_all[:, b * 4 + 2 + m : b * 4 + 3 + m],
                        o2scr[:],
                        mybir.AxisListType.X,
                        mybir.AluOpType.add,
                    )
                    nc.vector.tensor_scalar_mul(
                        dscs[:, b * 4 + 2 + m : b * 4 + 3 + m],
                        docs_all[:, b * 4 + 2 + m : b * 4 + 3 + m],
                        1.0 / (S * WSCALE))
                nc.vector.tensor_scalar_mul(
                    dscs[:, b * 4 : b * 4 + 2],
                    docs_all[:, b * 4 : b * 4 + 2], 1.0 / S)
                # scores run two examples behind: their inputs have been
                # ready for a full example, so the PE never stalls on them
                prevs.append((b, o1T))
                if len(prevs) > 2:
                    do_scores(*prevs.pop(0))
            # ---- tail: final scores interleaved with the head ----
            # head columns 0..BL-2 only need the first BL-1 doc vectors, so
            # they run before ex BL-1's eviction chain completes; their ph
            # psums reuse the (long-evicted) o1 psum banks.
            hpool = mctx.enter_context(tc.tile_pool(name="head", bufs=1))
            docs_v = dscs[:].rearrange("p (b k) -> p k b", k=4)
            fdw_v = fdw_sb[:].rearrange("p (k h) -> p k h", k=4)
            h16 = hpool.tile([128, MH * BL], F16)

            def head_cols(lo, hi):
                for m in range(MH):
                    ph = mm1_ps.tile([128, hi - lo], F32, tag=f"p1{m}",
                                     name=f"ph{m}")
                    for kc in range(4):
                        nc.tensor.matmul(
                            ph[:],
                            fdw_v[:, kc, m * 128 : (m + 1) * 128],
                            docs_v[:, kc, lo:hi],
                            start=(kc == 0),
                            stop=(kc == 3),
                        )
                    nc.scalar.activation(
                        h16[:, m * BL + lo : m * BL + hi],
                        ph[:],
                        AF.Relu,
                        bias=fdb_sb[:, m : m + 1],
                    )

            do_scores(*prevs.pop(0))
            head_cols(0, BL - 1)
            do_scores(*prevs.pop(0))
            head_cols(BL - 1, BL)

            po = dd_ps.tile([1, BL], F32, tag="sdd", name="po")
            for m in range(MH):
                nc.tensor.matmul(
                    po[:],
                    ffw_sb[:, m : m + 1],
                    h16[:, m * BL : (m + 1) * BL],
                    start=(m == 0),
                    stop=(m == MH - 1),
                )
            # sigmoid(x) = 1/(1+exp(-x)) — stays in the Exp table set
            sig_row = hpool.tile([1, BL], F32)
            nc.scalar.activation(sig_row[:], po[:], AF.Exp,
                                 bias=nffb_sb[0:1, 0:1], scale=-1.0)
            nc.vector.tensor_scalar_add(sig_row[:], sig_row[:], 1.0)
            nc.vector.reciprocal(sig_row[:], sig_row[:])

            # final output column: out[:, S] = sigmoid head values
            nc.sync.dma_start(
                out[:, S : S + 1],
                sig_row[0:1, :].rearrange("o (b s) -> o b s", b=BL),
            )


_NC_CACHE = None


def _get_nc():
    global _NC_CACHE
    if _NC_CACHE is None:
        nc = bacc.Bacc("TRN2", target_bir_lowering=False, debug=False,
                       num_devices=NCORES)
        with tile.TileContext(nc) as tc:
            _build_kernel(tc)
        nc.compile()
        _NC_CACHE = nc
    return _NC_CACHE


def kernel(output_1, output_2, mlp_w, mlp_b, fd_w, fd_b, ff_w, ff_b):
    output_1 = np.asarray(output_1, dtype=np.float32)
    output_2 = np.asarray(output_2, dtype=np.float32)
    mlp_w = np.asarray(mlp_w, dtype=np.float32)
    mlp_b = np.asarray(mlp_b, dtype=np.float32)
    fd_w = np.asarray(fd_w, dtype=np.float32)
    fd_b = np.asarray(fd_b, dtype=np.float32)
    ff_w = np.asarray(ff_w, dtype=np.float32)
    ff_b = np.asarray(ff_b, dtype=np.float32)

    # shard over batch; [*, S, V] -> [*, p, k, S] with V = k*128 + p
    def to_pks(x):
        return np.ascontiguousarray(
            x.reshape(NCORES, BL, S, KV, 128).transpose(0, 1, 4, 3, 2))

    x1 = to_pks(output_1).astype(np.float16).reshape(NCORES, BL, 128, KV * S)
    x2 = to_pks(output_2).astype(E4NP).reshape(NCORES, BL, 128, KV * S)

    wt = np.ascontiguousarray(
        mlp_w.T.reshape(KV, 128, H).transpose(1, 0, 2))        # [p, k, H]
    w16 = wt.astype(np.float16).reshape(128, KV * H)
    w8 = (wt * WSCALE).astype(E4NP).reshape(128, KV * H)

    mlpb = np.ascontiguousarray(mlp_b.reshape(MH, 128).T)      # [128, MH]
    mlpb64 = np.ascontiguousarray(mlpb * WSCALE)
    fdw = np.ascontiguousarray(
        fd_w.T.reshape(4, 128, H).transpose(1, 0, 2)
    ).reshape(128, 4 * H).astype(np.float16)
    fdb = np.ascontiguousarray(fd_b.reshape(MH, 128).T)
    ffw = np.ascontiguousarray(
        ff_w.reshape(MH, 128).T).astype(np.float16)            # [128, MH]
    nffb = np.ascontiguousarray(-ff_b.reshape(1, 1))

    in_maps = [
        dict(x1=x1[c], x2=x2[c], w16=w16, w8=w8, mlpb=mlpb, mlpb64=mlpb64,
             fdw=fdw, fdb=fdb, ffw=ffw, nffb=nffb)
        for c in range(NCORES)
    ]
    global _LAST_IN_MAPS
    _LAST_IN_MAPS = in_maps
    nc = _get_nc()
    res = bass_utils.run_bass_kernel_spmd(nc, in_maps, core_ids=list(range(NCORES)))
    att = np.concatenate([res.results[c]["out"] for c in range(NCORES)], axis=0)
    return np.ascontiguousarray(att.T)  # [S+1, B]
